# revision 1
# baseline (speedup 1.0000x reference)
"""MultiHeadAttention Trainium2 kernel.

Sharding: 8 cores = 4 batches x 2 head-groups (8 heads each).
Each core computes, for its (batch b, head group gi):
  Q = q[b] @ Wq[:, gi*512:+512] + bq_g        (and same fc applied to k, v)
  per head: softmax(QK^T/8 with mask) @ V
  partial_out = attn @ Wo[gi*512:+512, :]
Host sums the two partial outputs per batch and adds b_o.

Device layout notes (per core):
  - inputs arrive TRANSPOSED: xT [1024, seq] so projections need no transposes
  - Q^T, K^T stored [128, 4, seq] bf16 (partition = d within d-tile; head g
    occupies partitions 64*(g%2).. of d-tile g//2 -> natural head pairing)
  - scores computed transposed S^T[sk, sq] with two heads packed in the PE
    array via tile_position row tiling (K=64 each)
  - softmax without max-subtraction (scores bounded ~|6| after 1/8 scaling)
  - mask applied multiplicatively AFTER exp (notmask in {0,1} bf16)
  - attn@V uses lhsT=[V_head | ones] (M=65): row 64 accumulates the softmax
    denominator for free
  - normalize via exp(-ln(sum)) on ACT + gpsimd partition_broadcast (custom
    DVE reciprocal ops are broken on HW via this runtime path)
"""

import sys

import numpy as np
import ml_dtypes

try:
    import concourse.bass as bass  # noqa: F401
except ImportError:  # pragma: no cover
    for _p in ("/opt/trn_rl_repo", "/root/.axon_site/_ro/trn_rl_repo"):
        if _p not in sys.path:
            sys.path.insert(0, _p)
    import concourse.bass as bass  # noqa: F401

import concourse.tile as tile
from concourse import bacc, mybir
from concourse.bass_utils import run_bass_kernel_spmd

BF16 = ml_dtypes.bfloat16

D_MODEL = 1024
N_HEADS = 16
BATCH = 4
SEQ = 2048
DH = 64           # head dim
HG = 8            # heads per core
DG = HG * DH      # 512, projected dim per core

F32 = mybir.dt.float32
F32R = mybir.dt.float32r
BF16D = mybir.dt.bfloat16


def build_nc(seq=SEQ):
    """Build the per-core SPMD Bass program."""
    assert seq % 512 == 0
    NT = seq // 128       # sk tiles
    NC_ = seq // 512      # sq chunks
    NST = seq // 128      # s tiles for V / out rows

    nc = bacc.Bacc(None, target_bir_lowering=False)

    xqT = nc.dram_tensor("xqT", [D_MODEL, seq], F32R, kind="ExternalInput")
    xkT = nc.dram_tensor("xkT", [D_MODEL, seq], F32R, kind="ExternalInput")
    xvT = nc.dram_tensor("xvT", [D_MODEL, seq], F32R, kind="ExternalInput")
    wq = nc.dram_tensor("wq", [D_MODEL, DG], F32R, kind="ExternalInput")
    bq = nc.dram_tensor("bq", [DG], F32, kind="ExternalInput")
    wo = nc.dram_tensor("wo", [DG, D_MODEL], BF16D, kind="ExternalInput")
    # notmask, transposed + tiled: [pair, c, t, p(sk in tile), h(in pair), j(sq in chunk)]
    nm = nc.dram_tensor("nm", [4, NC_, NT, 128, 2, 512], BF16D, kind="ExternalInput")
    out = nc.dram_tensor("out", [seq, D_MODEL], F32, kind="ExternalOutput")

    EXP = mybir.ActivationFunctionType.Exp
    LN = mybir.ActivationFunctionType.Ln
    IDENT = mybir.ActivationFunctionType.Identity

    with tile.TileContext(nc) as tc:
        with tc.tile_pool(name="persist", bufs=1) as persist:
            qt_sb = persist.tile([128, 4, seq], BF16D, name="qt_sb")
            kt_sb = persist.tile([128, 4, seq], BF16D, name="kt_sb")
            v_sb = persist.tile([128, NST, HG, DH + 1], BF16D, name="v_sb")
            wo_sb = persist.tile([128, 4, D_MODEL], BF16D, name="wo_sb")
            attnT = persist.tile([128, 4, seq], BF16D, name="attnT")
            bq_sb = persist.tile([128, 4], F32, name="bq_sb")
            bqrep = persist.tile([128, HG, DH], F32, name="bqrep")

            # ---------------- Phase A: projections ----------------
            with tc.tile_pool(name="xpool", bufs=12) as xpool, \
                 tc.tile_pool(name="wqpool", bufs=1) as wqp, \
                 tc.tile_pool(name="projps", bufs=2, space="PSUM") as projps:

                wq_sb = wqp.tile([128, 8, DG], F32R, name="wq_sb")
                nc.sync.dma_start(out=wq_sb, in_=wq.rearrange("(n p) m -> p n m", p=128))
                nc.sync.dma_start(out=bq_sb, in_=bq.rearrange("(t p) -> p t", p=128))
                _bqap = bq[:].rearrange("(g e) -> g e", g=HG)
                nc.gpsimd.dma_start(out=bqrep, in_=bass.AP(
                    tensor=_bqap.tensor, offset=_bqap.offset,
                    ap=[[0, 128]] + [list(d) for d in _bqap.ap]))
                nc.sync.dma_start(out=wo_sb, in_=wo.rearrange("(n p) m -> p n m", p=128))
                nc.vector.memset(v_sb[:, :, :, DH:DH + 1], 1.0)

                def load_x(xh, pref):
                    ts_ = []
                    for db in range(8):
                        xt = xpool.tile([128, seq], F32R, name=f"{pref}{db}", tag="x")
                        nc.sync.dma_start(out=xt, in_=xh[db * 128:(db + 1) * 128, :])
                        ts_.append(xt)
                    return ts_

                # Q^T and K^T: out[d_tile, sq] accumulated over D blocks
                for xh, dst in ((xqT, qt_sb), (xkT, kt_sb)):
                    xts = load_x(xh, "xq" if dst is qt_sb else "xk")
                    for dt in range(4):
                        for h0 in range(0, seq, 1024):
                            cw = min(1024, seq - h0)
                            ps = projps.tile([128, cw], F32, name="pps", tag="pps")
                            for db in range(8):
                                for j0 in range(0, cw, 512):
                                    nc.tensor.matmul(
                                        ps[:, j0:j0 + 512],
                                        wq_sb[:, db, dt * 128:(dt + 1) * 128],
                                        xts[db][:, h0 + j0: h0 + j0 + 512],
                                        start=(db == 0), stop=(db == 7),
                                    )
                            nc.scalar.activation(
                                dst[:, dt, h0:h0 + cw], ps,
                                IDENT, bias=bq_sb[:, dt:dt + 1], scale=1.0,
                            )
                # V: out[s_tile, d] accumulated over D blocks
                xts = load_x(xvT, "xv")
                for st in range(NST):
                    ps = projps.tile([128, DG], F32, name="pps", tag="pps")
                    for db in range(8):
                        nc.tensor.matmul(
                            ps, xts[db][:, st * 128:(st + 1) * 128], wq_sb[:, db, :],
                            start=(db == 0), stop=(db == 7),
                        )
                    nc.vector.tensor_add(
                        v_sb[:, st, :, 0:DH],
                        ps[:, :].rearrange("p (g e) -> p g e", g=HG),
                        bqrep,
                    )

            # ---------------- Phase B: attention ----------------
            with tc.tile_pool(name="nmp", bufs=10) as nmp, \
                 tc.tile_pool(name="probsp", bufs=4) as probsp, \
                 tc.tile_pool(name="rsbp", bufs=2) as rsbp, \
                 tc.tile_pool(name="rrepp", bufs=2) as rrepp, \
                 tc.tile_pool(name="tmpp", bufs=2) as tmpp, \
                 tc.tile_pool(name="osbp", bufs=3) as osbp, \
                 tc.tile_pool(name="spairp", bufs=2, space="PSUM") as spairp, \
                 tc.tile_pool(name="accp", bufs=1, space="PSUM") as accp, \
                 tc.tile_pool(name="outpsp", bufs=2, space="PSUM") as outpsp:

                for c in range(NC_):
                    cs = slice(c * 512, (c + 1) * 512)
                    for pr in range(4):
                        acc = accp.tile([DH + 1, 2, 512], F32, name="acc", tag="acc")
                        for t in range(NT):
                            nmt = nmp.tile([128, 2, 512], BF16D, name="nmt", tag="nmt")
                            nc.sync.dma_start(out=nmt, in_=nm[pr, c, t])
                            spair = spairp.tile([128, 2, 512], F32, name="spair", tag="spair")
                            tc_cols = slice(t * 128, (t + 1) * 128)
                            nc.tensor.matmul(
                                spair[:, 0, :], kt_sb[0:64, pr, tc_cols],
                                qt_sb[0:64, pr, cs], start=True, stop=True,
                                tile_position=(0, 0),
                            )
                            nc.tensor.matmul(
                                spair[:, 1, :], kt_sb[64:128, pr, tc_cols],
                                qt_sb[64:128, pr, cs], start=True, stop=True,
                                tile_position=(64, 0),
                            )
                            probs = probsp.tile([128, 2, 512], BF16D, name="probs", tag="probs")
                            nc.scalar.activation(probs, spair, EXP, scale=0.125)
                            nc.vector.tensor_mul(probs, probs, nmt)
                            for h in range(2):
                                nc.tensor.matmul(
                                    acc[:, h, :], v_sb[:, t, 2 * pr + h, :],
                                    probs[:, h, :], start=(t == 0), stop=(t == NT - 1),
                                )
                        # normalize this pair's chunk
                        lnt = rsbp.tile([1, 2, 512], F32, name="lnt", tag="lnt")
                        nc.scalar.activation(lnt, acc[DH:DH + 1, :, :], LN, scale=1.0)
                        rsb = rsbp.tile([1, 2, 512], F32, name="rsb", tag="rsb")
                        nc.scalar.activation(rsb, lnt, EXP, scale=-1.0)
                        rrep = rrepp.tile([64, 2, 512], F32, name="rrep", tag="rrep")
                        nc.gpsimd.partition_broadcast(rrep, rsb)
                        tmpt = tmpp.tile([64, 2, 512], BF16D, name="tmpt", tag="tmpt")
                        nc.vector.tensor_mul(tmpt, acc[0:DH, :, :], rrep)
                        nc.sync.dma_start(out=attnT[0:64, pr, cs], in_=tmpt[:, 0, :])
                        nc.sync.dma_start(out=attnT[64:128, pr, cs], in_=tmpt[:, 1, :])
                    # output projection for this sq chunk
                    for k in range(4):
                        row = c * 512 + k * 128
                        for dch in range(2):
                            ops = outpsp.tile([128, 512], F32, name="ops", tag="ops")
                            for pr in range(4):
                                nc.tensor.matmul(
                                    ops, attnT[:, pr, row:row + 128],
                                    wo_sb[:, pr, dch * 512:(dch + 1) * 512],
                                    start=(pr == 0), stop=(pr == 3),
                                )
                            osb = osbp.tile([128, 512], F32, name="osb", tag="osb")
                            nc.vector.tensor_copy(osb, ops)
                            nc.sync.dma_start(
                                out=out[row:row + 128, dch * 512:(dch + 1) * 512], in_=osb
                            )

    nc.compile()
    return nc


_NC_CACHE = {}


def _get_nc(seq=SEQ):
    if seq not in _NC_CACHE:
        _NC_CACHE[seq] = build_nc(seq)
    return _NC_CACHE[seq]


def make_core_inputs(q, k, v, mask, W_q, b_q, W_o, seq=SEQ):
    """Build the 8 per-core input maps (host-side shard + layout)."""
    NT = seq // 128
    NC_ = seq // 512
    in_maps = []
    notm_all = (~np.asarray(mask)).astype(BF16)  # [B, 16, sq, sk]
    for core in range(8):
        b, gi = divmod(core, 2)
        cols = slice(gi * DG, (gi + 1) * DG)
        xqT = np.ascontiguousarray(np.asarray(q[b], np.float32).T)
        xkT = np.ascontiguousarray(np.asarray(k[b], np.float32).T)
        xvT = np.ascontiguousarray(np.asarray(v[b], np.float32).T)
        wqc = np.ascontiguousarray(np.asarray(W_q, np.float32)[:, cols])
        bqc = np.ascontiguousarray(np.asarray(b_q, np.float32)[cols])
        woc = np.ascontiguousarray(np.asarray(W_o, np.float32)[cols, :]).astype(BF16)
        nmc = notm_all[b, gi * HG:(gi + 1) * HG]  # [8, sq, sk] bf16
        # -> [pair, c, t, p, h, j]
        nmc = np.ascontiguousarray(
            nmc.reshape(4, 2, NC_, 512, NT, 128).transpose(0, 2, 4, 5, 1, 3)
        )
        in_maps.append({
            "xqT": xqT, "xkT": xkT, "xvT": xvT,
            "wq": wqc, "bq": bqc, "wo": woc, "nm": nmc,
        })
    return in_maps


def kernel(q, k, v, mask, W_q, b_q, W_o, b_o):
    nc = _get_nc(SEQ)
    in_maps = make_core_inputs(q, k, v, mask, W_q, b_q, W_o, SEQ)
    res = run_bass_kernel_spmd(nc, in_maps, core_ids=list(range(8)))
    out = np.empty((BATCH, SEQ, D_MODEL), np.float32)
    bo = np.asarray(b_o, np.float32)
    for b in range(BATCH):
        out[b] = res.results[2 * b]["out"] + res.results[2 * b + 1]["out"] + bo
    return out



# revision 42
# speedup vs baseline: 1.5143x; 1.5143x over previous
"""MultiHeadAttention Trainium2 kernel (software-pipelined).

Sharding: 8 cores = 4 batches x 2 head-groups (8 heads each).
Each core computes, for its (batch b, head group gi):
  Q = q[b] @ Wq[:, gi*512:+512] + bq_g        (same fc applied to k and v)
  per head: softmax(QK^T/8 with mask) @ V
  partial_out = attn @ Wo[gi*512:+512, :]
Host sums the two partial outputs per batch and adds b_o.

Design (per core), driven by the CoreSim cost model:
  - ACT exp of all 33.5M scores (~266us) and PE matmul columns (~285us) are
    the two near-equal hard floors; everything is scheduled so both engines
    stream continuously from ~18us in.
  - 16 "units" (pr-major: unit u = head-pair pr * 4 + sq-chunk c). Per unit:
    16 score tiles S^T[sk 128, 2 heads, sq 512] -> ACT exp -> DVE mask-mul.
    The unit's 8 attnV accumulation chains (probs^T stationary, [V|ones]
    moving, N=65 -> half the PE cost of the V-stationary form) run at the
    START of the next unit's emission; the denominator rides in output
    column 64, so normalization is a per-partition (per-query) DVE
    tensor_scalar_mul after a tiny ACT ln/exp(-x) reciprocal.
  - All projections (Q^T, K^T, V) are "filler" chains spread just-in-time
    through the units so the PE never idles while ACT streams exps.
  - attn[sq, d] transposed back to attnT[d, sq] with PE transposes (f32,
    sharing the [128,512] f32 PSUM tag); Pool does every PSUM->SBUF move
    (+bias) so ACT/DVE stay free.
  - bf16 on the PE everywhere; mask shipped bf16 {0,1} (DVE 2x mode needs
    2-byte dtypes); output partials bf16.
  - PSUM exactly 8 banks: spair 2x2 + acc0 + acc1 + shared genps 2.
  - nm mask streamed in eighths ([128,2t,2h,512]) on the DVE queue with a
    3-slot lead; buffer rings sized so no engine ever waits on a ring wrap.
"""

import sys

import numpy as np
import ml_dtypes

try:
    import concourse.bass as bass  # noqa: F401
except ImportError:  # pragma: no cover
    for _p in ("/opt/trn_rl_repo", "/root/.axon_site/_ro/trn_rl_repo"):
        if _p not in sys.path:
            sys.path.insert(0, _p)
    import concourse.bass as bass  # noqa: F401

import contextlib

import concourse.tile as tile
from concourse import bacc, mybir
from concourse.bass_utils import run_bass_kernel_spmd

BF16 = ml_dtypes.bfloat16

D_MODEL = 1024
N_HEADS = 16
BATCH = 4
SEQ = 2048
DH = 64           # head dim
HG = 8            # heads per core
DG = HG * DH      # 512, projected dim per core

F32 = mybir.dt.float32
BF16D = mybir.dt.bfloat16


def build_nc(seq=SEQ):
    """Build the per-core SPMD Bass program."""
    assert seq == 2048, "schedule is hardcoded for seq=2048"
    NT = seq // 128       # 16 sk tiles
    NC_ = seq // 512      # 4 sq chunks

    nc = bacc.Bacc(None, target_bir_lowering=False)

    xqT = nc.dram_tensor("xqT", [D_MODEL, seq], BF16D, kind="ExternalInput")
    xkT = nc.dram_tensor("xkT", [D_MODEL, seq], BF16D, kind="ExternalInput")
    xvT = nc.dram_tensor("xvT", [D_MODEL, seq], BF16D, kind="ExternalInput")
    wq = nc.dram_tensor("wq", [D_MODEL, DG], BF16D, kind="ExternalInput")
    bq = nc.dram_tensor("bq", [DG], F32, kind="ExternalInput")
    wo = nc.dram_tensor("wo", [DG, D_MODEL], BF16D, kind="ExternalInput")
    ident_in = nc.dram_tensor("ident_in", [128, 128], F32, kind="ExternalInput")
    # notmask: [pair, c, t, p(sk in tile), h(in pair), j(sq in chunk)]
    nm = nc.dram_tensor("nm", [4, NC_, NT, 128, 2, 512], BF16D, kind="ExternalInput")
    out = nc.dram_tensor("out", [seq, D_MODEL], BF16D, kind="ExternalOutput")

    EXP = mybir.ActivationFunctionType.Exp
    LN = mybir.ActivationFunctionType.Ln

    es = contextlib.ExitStack()
    with tile.TileContext(nc) as tc, es:
        persist = es.enter_context(tc.tile_pool(name="persist", bufs=1))
        attnT = persist.tile([128, 4, seq], BF16D, name="attnT")
        ident = persist.tile([128, 128], F32, name="ident")
        bq_sb = persist.tile([128, 4], F32, name="bq_sb")
        bqrep = persist.tile([128, HG, DH], F32, name="bqrep")

        wqp = es.enter_context(tc.tile_pool(name="wqp", bufs=1))
        wq_sb = wqp.tile([128, 8, DG], BF16D, name="wq_sb")

        qtp = es.enter_context(tc.tile_pool(name="qtp", bufs=2))
        ktp = es.enter_context(tc.tile_pool(name="ktp", bufs=2))
        vprp = es.enter_context(tc.tile_pool(name="vprp", bufs=2))
        xvp = es.enter_context(tc.tile_pool(name="xvp", bufs=8))
        xkp = es.enter_context(tc.tile_pool(name="xkp", bufs=2))
        xqp = es.enter_context(tc.tile_pool(name="xqp", bufs=4))
        nmp = es.enter_context(tc.tile_pool(name="nmp", bufs=3))
        probsp = es.enter_context(tc.tile_pool(name="probsp", bufs=22))
        densp = es.enter_context(tc.tile_pool(name="densp", bufs=2))
        attnnp = es.enter_context(tc.tile_pool(name="attnnp", bufs=1))
        osbp = es.enter_context(tc.tile_pool(name="osbp", bufs=2))
        spairp = es.enter_context(tc.tile_pool(name="spairp", bufs=2, space="PSUM"))
        accp = es.enter_context(tc.tile_pool(name="accp", bufs=1, space="PSUM"))
        genps = es.enter_context(tc.tile_pool(name="genps", bufs=2, space="PSUM"))

        # ---------- initial DMAs (SP queue, in priority order) ----------
        def load_wq_cols(dt):
            nc.sync.dma_start(
                out=wq_sb[:, :, dt * 128:(dt + 1) * 128],
                in_=wq[:, dt * 128:(dt + 1) * 128].rearrange("(n p) m -> p n m", p=128))
        load_wq_cols(0)
        nc.sync.dma_start(out=bq_sb, in_=bq.rearrange("(t p) -> p t", p=128))
        _bqap = bq[:].rearrange("(g e) -> g e", g=HG)
        nc.gpsimd.dma_start(out=bqrep, in_=bass.AP(
            tensor=_bqap.tensor, offset=_bqap.offset,
            ap=[[0, 128]] + [list(d) for d in _bqap.ap]))
        nc.sync.dma_start(out=ident, in_=ident_in[:, :])

        xq_t = {}   # c -> tile [128, 8, 512]

        def load_xq_chunk(c):
            xt = xqp.tile([128, 8, 512], BF16D, name=f"xq{c}", tag="xqc")
            nc.sync.dma_start(
                out=xt,
                in_=xqT[:, c * 512:(c + 1) * 512].rearrange("(n p) m -> p n m", p=128))
            xq_t[c] = xt

        xk_t = []
        for half in range(2):
            xh = xkp.tile([128, 4, seq], BF16D, name=f"xk{half}", tag="xk")
            nc.sync.dma_start(
                out=xh, in_=xkT[half * 512:(half + 1) * 512].rearrange(
                    "(n p) m -> p n m", p=128))
            xk_t.append(xh)
            if half == 0:
                load_xq_chunk(0)

        # nm eighths: tile [128, 2(t), 2(h), 512]; SP queue, issued inside the
        # slot stream so the ring wait always lands on long-finished muls.
        nm_tiles = {}   # (u, i) -> tile

        def fetch_nm_eighth(u, i):
            upr, uc = u // 4, u % 4
            t0 = 2 * i
            nmq = nmp.tile([128, 2, 2, 512], BF16D, name="nmq", tag="nmq")
            nc.sync.dma_start(
                out=nmq, in_=nm[upr, uc, t0:t0 + 2].rearrange("t p h j -> p t h j"))
            nm_tiles[(u, i)] = nmq

        # ---------- filler chains as ~430ns PE quanta ----------
        # The tile framework encodes deps as per-engine counting semaphores,
        # so each exp effectively waits for ALL PE work emitted before its
        # score pair. Fillers are therefore emitted as small fixed-cost
        # quanta paced so every slot carries a uniform PE load.
        kt_t = {}
        qt_t = {}
        vpr_t = {}

        def KT(dt, q):
            # 4 quanta of 2 N=512 matmuls; Pool bias-add rides on the last
            st = {}
            cols = slice(q * 512, (q + 1) * 512)

            def part(i):
                def f():
                    if i == 0:
                        if dt not in kt_t:
                            kt_t[dt] = ktp.tile([128, seq], BF16D,
                                                name=f"kt{dt}", tag="kt")
                        st["ps"] = genps.tile([128, 512], F32, name="pps", tag="pps")
                    for db in (2 * i, 2 * i + 1):
                        nc.tensor.matmul(
                            st["ps"], wq_sb[:, db, dt * 128:(dt + 1) * 128],
                            xk_t[db // 4][:, db % 4, cols], start=(db == 0), stop=(db == 7))
                    if i == 3:
                        nc.vector.tensor_scalar_add(
                            kt_t[dt][:, cols], st["ps"], bq_sb[:, dt:dt + 1])
                return f
            return [(430, part(i)) for i in range(4)]

        def QT(dt, c):
            st = {}
            cols = slice(c * 512, (c + 1) * 512)

            def part(i):
                def f():
                    if i == 0:
                        if dt not in qt_t:
                            qt_t[dt] = qtp.tile([128, seq], BF16D,
                                                name=f"qt{dt}", tag="qt")
                        st["ps"] = genps.tile([128, 512], F32, name="pps", tag="pps")
                    for db in (2 * i, 2 * i + 1):
                        nc.tensor.matmul(
                            st["ps"], wq_sb[:, db, dt * 128:(dt + 1) * 128],
                            xq_t[c][:, db, :], start=(db == 0), stop=(db == 7))
                    if i == 3:
                        nc.vector.tensor_scalar_add(
                            qt_t[dt][:, cols], st["ps"], bq_sb[:, dt:dt + 1])
                return f
            return [(430, part(i)) for i in range(4)]

        def V(st_, pr):
            def f():
                if pr not in vpr_t:
                    vpr_t[pr] = vprp.tile([128, NT, 2, DH + 1], BF16D,
                                          name=f"vpr{pr}", tag="vpr")
                    nc.vector.memset(vpr_t[pr][:, :, :, DH:DH + 1], 1.0)
                ps = genps.tile([128, 512], F32, name="pps", tag="pps")
                for db in range(8):
                    nc.tensor.matmul(ps[:, 0:128],
                                     xv_t[db][:, st_ * 128:(st_ + 1) * 128],
                                     wq_sb[:, db, pr * 128:(pr + 1) * 128],
                                     start=(db == 0), stop=(db == 7))
                nc.vector.tensor_add(
                    vpr_t[pr][:, st_, :, 0:DH],
                    ps[:, 0:128].rearrange("p (h e) -> p h e", h=2),
                    bqrep[:, 2 * pr:2 * pr + 2, :])
            return [(430, f)]

        wo_holder = {}

        def OP(c, j):
            k, dch = j // 2, j % 2
            row = c * 512 + k * 128
            st = {}

            def part(i):
                def f():
                    if i == 0:
                        st["ps"] = genps.tile([128, 512], F32, name="pps", tag="pps")
                    for pr in (2 * i, 2 * i + 1):
                        nc.tensor.matmul(st["ps"], attnT[:, pr, row:row + 128],
                                         wo_holder["wo_t"][:, 2 * pr + dch, :],
                                         start=(pr == 0), stop=(pr == 3))
                    if i == 1:
                        osb = osbp.tile([128, 512], BF16D, name="osb", tag="osb")
                        nc.vector.tensor_copy(osb, st["ps"])
                        nc.sync.dma_start(
                            out=out[row:row + 128, dch * 512:(dch + 1) * 512],
                            in_=osb)
                return f
            return [(430, part(0)), (430, part(1))]

        # ---------- attention pipeline pieces ----------
        probs_store = {}  # u -> list of 16 probs tiles
        acc_store = {}    # u -> (acc0, acc1)

        def attn_chain(u, ci):
            def f():
                h, sub = ci // 4, ci % 4
                acch = acc_store[u][h]
                pr = u // 4
                for t in range(NT):
                    nc.tensor.matmul(
                        acch[:, sub, :],
                        probs_store[u][t][:, h, sub * 128:(sub + 1) * 128],
                        vpr_t[pr][:, t, h, :],
                        start=(t == 0), stop=(t == NT - 1))
            return [(440, f)]

        def finish_unit(u):
            pr, c = u // 4, u % 4
            cs = slice(c * 512, (c + 1) * 512)
            st = {}

            def recip():
                acc0, acc1 = acc_store[u]
                dens = densp.tile([128, 8], F32, name="dens", tag="dens")
                nc.vector.tensor_copy(
                    dens[:, 0:4], acc0[:, :, DH:DH + 1].rearrange("p a b -> p (a b)"))
                nc.vector.tensor_copy(
                    dens[:, 4:8], acc1[:, :, DH:DH + 1].rearrange("p a b -> p (a b)"))
                junk = densp.tile([128, 8], F32, name="junk", tag="junk")
                for j in range(8):
                    nc.gpsimd.normalize_recip(
                        junk[:, j:j + 1], dens[:, j:j + 1], dens[:, j:j + 1])
                st["dens"] = dens
                st["attn_n"] = attnnp.tile([128, 4, 2, DH], F32,
                                           name="attn_n", tag="attn_n")
                st["trans"] = genps.tile([128, 512], F32, name="pps", tag="pps")

            def norm(h):
                def f():
                    acch = acc_store[u][h]
                    for sub in range(4):
                        nc.vector.tensor_scalar_mul(
                            st["attn_n"][:, sub, h, :], acch[:, sub, 0:DH],
                            st["dens"][:, 4 * h + sub:4 * h + sub + 1])
                return f

            def trans4():
                # one transpose per sq-subtile covering both heads:
                # [128 sq, 128 d-pair] -> [128, 128] at PSUM partition 0
                # (HW requires transpose outputs to start at partition 0)
                tr4 = st["trans"][:, :].rearrange("p (a b) -> p a b", a=4)
                for sub in range(4):
                    nc.tensor.transpose(
                        tr4[:, sub, :],
                        st["attn_n"][:, sub, :, :].rearrange("p h e -> p (h e)"),
                        ident)

            def copy_out():
                nc.vector.tensor_copy(attnT[:, pr, cs], st["trans"])
                del probs_store[u]
            return [(60, recip), (40, norm(0)), (40, norm(1)),
                    (430, trans4), (60, copy_out)]

        # ---------- per-unit filler schedules (lists of quanta) ----------
        def qsum(*gens):
            out_ = []
            for g in gens:
                out_ += g
            return out_

        PRO_QT = QT(0, 0)
        PRO_KT = KT(0, 0)
        POST = {
            0: qsum(KT(0, 1), KT(0, 2), KT(0, 3))
               + [(cst, f, 9) for cst, f in QT(0, 1)]
               + [(cst, f, 10) for g in [V(s, 0) for s in range(16)] for cst, f in g],
            1: qsum(KT(1, 0)) + [(cst, f, 9) for cst, f in QT(0, 2)],
            2: qsum(KT(1, 1), KT(1, 2), KT(1, 3))
               + [(cst, f, 9) for cst, f in QT(0, 3)],
            3: qsum(QT(1, 0), *[V(s, 1) for s in range(8)]),
            4: qsum(KT(2, 0), KT(2, 1), QT(1, 1), *[V(s, 1) for s in range(8, 16)]),
            5: qsum(KT(2, 2), KT(2, 3), QT(1, 2)),
            6: qsum(QT(1, 3), *[V(s, 2) for s in range(8)]),
            7: qsum(QT(2, 0), *[V(s, 2) for s in range(8, 16)]),
            8: qsum(KT(3, 0), KT(3, 1), QT(2, 1)),
            9: qsum(KT(3, 2), KT(3, 3), QT(2, 2)),
            10: qsum(QT(2, 3), *[V(s, 3) for s in range(8)]),
            11: qsum(QT(3, 0), *[V(s, 3) for s in range(8, 16)]),
            12: qsum(QT(3, 1)),
            13: qsum(QT(3, 2), *[OP(0, j) for j in range(8)]),
            14: qsum(QT(3, 3), *[OP(1, j) for j in range(8)]),
            15: qsum(*[OP(2, j) for j in range(8)]),
        }
        TAIL = qsum(*[OP(3, j) for j in range(8)])

        # ---------- main loop ----------
        # xv immediately after xk so u0's V fillers are never DMA-blocked
        xv_t = []
        for db in range(8):
            xt = xvp.tile([128, seq], BF16D, name=f"xv{db}", tag="xv")
            nc.sync.dma_start(out=xt, in_=xvT[db * 128:(db + 1) * 128, :])
            xv_t.append(xt)
        fetch_nm_eighth(0, 0)
        fetch_nm_eighth(0, 1)
        fetch_nm_eighth(0, 2)
        # Warm the PE p-state (full clock needs ~3us of continuous busy) and
        # keep it busy until each DMA lands; results are never read. Uses only
        # the dt0 columns of wq (the first wq DMA). The first K/Q chains are
        # interleaved with the warmup bursts to match xq-c0/xk-a/xk-b arrival.
        def warm(n):
            for w in range(n):
                wps = genps.tile([128, 512], F32, name="pps", tag="pps")
                nc.tensor.matmul(wps[:, 0:128], wq_sb[:, 0, 0:128],
                                 wq_sb[:, w % 8, 0:128], start=True, stop=True)
        warm(165)
        for _, f in PRO_QT:
            f()
        for _, f in PRO_KT[:2]:
            f()
        warm(85)
        for _, f in PRO_KT[2:]:
            f()

        for u in range(16):
            pr, c = u // 4, u % 4
            cs = slice(c * 512, (c + 1) * 512)
            acc0 = accp.tile([128, 4, DH + 1], F32, name="acc0", tag="acc0")
            acc1 = accp.tile([128, 4, DH + 1], F32, name="acc1", tag="acc1")
            acc_store[u] = (acc0, acc1)
            probs_store[u] = [None] * NT

            extras = []
            if u > 0:
                for ci in range(8):
                    extras += attn_chain(u - 1, ci)
                extras += finish_unit(u - 1)
            extras += POST[u]
            extras = [x if len(x) == 3 else (x[0], x[1], 0) for x in extras]
            total_cost = sum(x[0] for x in extras) or 1
            done_cost = 0
            n_x = 0
            for t in range(NT):
                spair = spairp.tile([128, 2, 512], F32, name="spair", tag="spair")
                tcol = slice(t * 128, (t + 1) * 128)
                nc.tensor.matmul(
                    spair[:, 0, :], kt_t[pr][0:64, tcol], qt_t[pr][0:64, cs],
                    start=True, stop=True, tile_position=(0, 0))
                nc.tensor.matmul(
                    spair[:, 1, :], kt_t[pr][64:128, tcol], qt_t[pr][64:128, cs],
                    start=True, stop=True, tile_position=(64, 0))
                probs = probsp.tile([128, 2, 512], BF16D, name="probs", tag="probs")
                nc.scalar.activation(probs, spair, EXP, scale=0.125)
                if t == 8 and u in (0, 1, 2):
                    load_xq_chunk(u + 1)
                if t == 14 and u == 0:
                    load_wq_cols(1)
                if t == 2 and u == 2:
                    load_wq_cols(2)
                if t == 2 and u == 5:
                    load_wq_cols(3)
                if t == 10 and u == 11:
                    # wo rides in the recycled xq-c0 buffer; the ring orders its
                    # DMA after the last QT(*, 0) reads.
                    wt = xqp.tile([128, 8, 512], BF16D, name="wo_t", tag="xqc")
                    _wap = wo[:, :]
                    nc.sync.dma_start(
                        out=wt[:, :, :].rearrange("p (n d) b -> p n d b", n=4),
                        in_=bass.AP(tensor=_wap.tensor, offset=_wap.offset,
                                    ap=[[1024, 128], [131072, 4], [512, 2],
                                        [1, 512]]))
                    wo_holder["wo_t"] = wt
                # prefetch nm eighth (prologue pre-loads e0-e2 of u0)
                if t % 2 == 1:
                    i_next = (t + 5) // 2
                    nu, ni = (u, i_next) if i_next < 8 else (u + 1, i_next - 8)
                    if nu < 16:
                        fetch_nm_eighth(nu, ni)
                nmq = nm_tiles[(u, t // 2)]
                if u == 0:
                    # u0's nm stream lands late (behind the x loads); Pool
                    # muls keep the DVE movers un-gated during warm-up
                    nc.gpsimd.tensor_mul(probs, probs, nmq[:, t % 2])
                else:
                    nc.vector.tensor_mul(probs, probs, nmq[:, t % 2])
                probs_store[u][t] = probs
                # pace extras by ns budget: counting-sem position semantics
                # make exp pace == PE pace through everything emitted before
                # each score pair, so keep per-slot PE load uniform.
                budget = total_cost * (t + 1) // NT
                while (n_x < len(extras) and done_cost < budget
                       and extras[n_x][2] <= t):
                    done_cost += extras[n_x][0]
                    extras[n_x][1]()
                    n_x += 1
            while n_x < len(extras):
                done_cost += extras[n_x][0]
                extras[n_x][1]()
                n_x += 1

        # tail: last unit's chains + finish + last outproj
        for ci in range(8):
            for _, f in attn_chain(15, ci):
                f()
        for _, f in finish_unit(15):
            f()
        for _, f in TAIL:
            f()

    nc.compile()
    return nc


_NC_CACHE = {}


def _get_nc(seq=SEQ):
    if seq not in _NC_CACHE:
        _NC_CACHE[seq] = build_nc(seq)
    return _NC_CACHE[seq]


def make_core_inputs(q, k, v, mask, W_q, b_q, W_o, seq=SEQ):
    """Build the 8 per-core input maps (host-side shard + layout)."""
    NT = seq // 128
    NC_ = seq // 512
    in_maps = []
    notm_all = (~np.asarray(mask)).astype(BF16)  # [B, 16, sq, sk]
    ident = np.eye(128, dtype=np.float32)
    for core in range(8):
        b, gi = divmod(core, 2)
        cols = slice(gi * DG, (gi + 1) * DG)
        xqT = np.ascontiguousarray(np.asarray(q[b], np.float32).T).astype(BF16)
        xkT = np.ascontiguousarray(np.asarray(k[b], np.float32).T).astype(BF16)
        xvT = np.ascontiguousarray(np.asarray(v[b], np.float32).T).astype(BF16)
        wqc = np.ascontiguousarray(np.asarray(W_q, np.float32)[:, cols]).astype(BF16)
        bqc = np.ascontiguousarray(np.asarray(b_q, np.float32)[cols])
        woc = np.ascontiguousarray(np.asarray(W_o, np.float32)[cols, :]).astype(BF16)
        nmc = notm_all[b, gi * HG:(gi + 1) * HG]  # [8, sq, sk] bf16
        # heads (pair, h) x [sq, sk] -> [pair, c, t, p(sk), h, j(sq)]
        nmc = np.ascontiguousarray(
            nmc.reshape(4, 2, NC_, 512, NT, 128).transpose(0, 2, 4, 5, 1, 3)
        )
        in_maps.append({
            "xqT": xqT, "xkT": xkT, "xvT": xvT,
            "wq": wqc, "bq": bqc, "wo": woc, "nm": nmc, "ident_in": ident,
        })
    return in_maps


def kernel(q, k, v, mask, W_q, b_q, W_o, b_o):
    nc = _get_nc(SEQ)
    in_maps = make_core_inputs(q, k, v, mask, W_q, b_q, W_o, SEQ)
    res = run_bass_kernel_spmd(nc, in_maps, core_ids=list(range(8)))
    out = np.empty((BATCH, SEQ, D_MODEL), np.float32)
    bo = np.asarray(b_o, np.float32)
    for b in range(BATCH):
        out[b] = (res.results[2 * b]["out"].astype(np.float32)
                  + res.results[2 * b + 1]["out"].astype(np.float32) + bo)
    return out


# revision 48
# speedup vs baseline: 1.5819x; 1.0446x over previous
"""MultiHeadAttention Trainium2 kernel (software-pipelined).

Sharding: 8 cores = 4 batches x 2 head-groups (8 heads each).
Each core computes, for its (batch b, head group gi):
  Q = q[b] @ Wq[:, gi*512:+512] + bq_g        (same fc applied to k and v)
  per head: softmax(QK^T/8 with mask) @ V
  partial_out = attn @ Wo[gi*512:+512, :]
Host sums the two partial outputs per batch and adds b_o.

Design (per core), driven by the CoreSim cost model:
  - ACT exp of all 33.5M scores (~266us) and PE matmul columns (~285us) are
    the two near-equal hard floors; everything is scheduled so both engines
    stream continuously from ~18us in.
  - 16 "units" (pr-major: unit u = head-pair pr * 4 + sq-chunk c). Per unit:
    16 score tiles S^T[sk 128, 2 heads, sq 512] -> ACT exp -> DVE mask-mul.
    The unit's 8 attnV accumulation chains (probs^T stationary, [V|ones]
    moving, N=65 -> half the PE cost of the V-stationary form) run at the
    START of the next unit's emission; the denominator rides in output
    column 64, so normalization is a per-partition (per-query) DVE
    tensor_scalar_mul after a tiny ACT ln/exp(-x) reciprocal.
  - All projections (Q^T, K^T, V) are "filler" chains spread just-in-time
    through the units so the PE never idles while ACT streams exps.
  - attn[sq, d] transposed back to attnT[d, sq] with PE transposes (f32,
    sharing the [128,512] f32 PSUM tag); Pool does every PSUM->SBUF move
    (+bias) so ACT/DVE stay free.
  - bf16 on the PE everywhere; mask shipped bf16 {0,1} (DVE 2x mode needs
    2-byte dtypes); output partials bf16.
  - PSUM exactly 8 banks: spair 2x2 + acc0 + acc1 + shared genps 2.
  - nm mask streamed in eighths ([128,2t,2h,512]) on the DVE queue with a
    3-slot lead; buffer rings sized so no engine ever waits on a ring wrap.
"""

import sys

import numpy as np
import ml_dtypes

try:
    import concourse.bass as bass  # noqa: F401
except ImportError:  # pragma: no cover
    for _p in ("/opt/trn_rl_repo", "/root/.axon_site/_ro/trn_rl_repo"):
        if _p not in sys.path:
            sys.path.insert(0, _p)
    import concourse.bass as bass  # noqa: F401

import contextlib

import concourse.tile as tile
from concourse import bacc, mybir
from concourse.bass_utils import run_bass_kernel_spmd

BF16 = ml_dtypes.bfloat16

D_MODEL = 1024
N_HEADS = 16
BATCH = 4
SEQ = 2048
DH = 64           # head dim
HG = 8            # heads per core
DG = HG * DH      # 512, projected dim per core

F32 = mybir.dt.float32
BF16D = mybir.dt.bfloat16


def build_nc(seq=SEQ):
    """Build the per-core SPMD Bass program."""
    assert seq == 2048, "schedule is hardcoded for seq=2048"
    NT = seq // 128       # 16 sk tiles
    NC_ = seq // 512      # 4 sq chunks

    nc = bacc.Bacc(None, target_bir_lowering=False)

    xqT = nc.dram_tensor("xqT", [D_MODEL, seq], BF16D, kind="ExternalInput")
    xkT = nc.dram_tensor("xkT", [D_MODEL, seq], BF16D, kind="ExternalInput")
    xvT = nc.dram_tensor("xvT", [D_MODEL, seq], BF16D, kind="ExternalInput")
    wq = nc.dram_tensor("wq", [D_MODEL, DG], BF16D, kind="ExternalInput")
    bq = nc.dram_tensor("bq", [DG], F32, kind="ExternalInput")
    wo = nc.dram_tensor("wo", [DG, D_MODEL], BF16D, kind="ExternalInput")
    ident_in = nc.dram_tensor("ident_in", [128, 128], F32, kind="ExternalInput")
    # notmask: [pair, c, t, p(sk in tile), h(in pair), j(sq in chunk)]
    nm = nc.dram_tensor("nm", [4, NC_, NT, 128, 2, 512], BF16D, kind="ExternalInput")
    out = nc.dram_tensor("out", [seq, D_MODEL], BF16D, kind="ExternalOutput")

    EXP = mybir.ActivationFunctionType.Exp
    LN = mybir.ActivationFunctionType.Ln

    es = contextlib.ExitStack()
    with tile.TileContext(nc) as tc, es:
        persist = es.enter_context(tc.tile_pool(name="persist", bufs=1))
        attnT = persist.tile([128, 4, seq], BF16D, name="attnT")
        ident = persist.tile([128, 128], F32, name="ident")
        bq_sb = persist.tile([128, 4], F32, name="bq_sb")
        bqrep = persist.tile([128, HG, DH], F32, name="bqrep")

        wqp = es.enter_context(tc.tile_pool(name="wqp", bufs=1))
        wq_sb = wqp.tile([128, 8, DG], BF16D, name="wq_sb")

        qtp = es.enter_context(tc.tile_pool(name="qtp", bufs=2))
        ktp = es.enter_context(tc.tile_pool(name="ktp", bufs=2))
        vprp = es.enter_context(tc.tile_pool(name="vprp", bufs=2))
        xvp = es.enter_context(tc.tile_pool(name="xvp", bufs=8))
        xkp = es.enter_context(tc.tile_pool(name="xkp", bufs=2))
        xqp = es.enter_context(tc.tile_pool(name="xqp", bufs=4))
        nmp = es.enter_context(tc.tile_pool(name="nmp", bufs=3))
        probsp = es.enter_context(tc.tile_pool(name="probsp", bufs=22))
        densp = es.enter_context(tc.tile_pool(name="densp", bufs=2))
        attnnp = es.enter_context(tc.tile_pool(name="attnnp", bufs=1))
        osbp = es.enter_context(tc.tile_pool(name="osbp", bufs=2))
        spairp = es.enter_context(tc.tile_pool(name="spairp", bufs=2, space="PSUM"))
        accp = es.enter_context(tc.tile_pool(name="accp", bufs=1, space="PSUM"))
        genps = es.enter_context(tc.tile_pool(name="genps", bufs=2, space="PSUM"))

        # ---------- initial DMAs (SP queue, in priority order) ----------
        def load_wq_cols(dt):
            nc.sync.dma_start(
                out=wq_sb[:, :, dt * 128:(dt + 1) * 128],
                in_=wq[:, dt * 128:(dt + 1) * 128].rearrange("(n p) m -> p n m", p=128))
        load_wq_cols(0)
        nc.sync.dma_start(out=bq_sb, in_=bq.rearrange("(t p) -> p t", p=128))
        _bqap = bq[:].rearrange("(g e) -> g e", g=HG)
        nc.gpsimd.dma_start(out=bqrep, in_=bass.AP(
            tensor=_bqap.tensor, offset=_bqap.offset,
            ap=[[0, 128]] + [list(d) for d in _bqap.ap]))
        nc.sync.dma_start(out=ident, in_=ident_in[:, :])

        xq_t = {}   # c -> tile [128, 8, 512]

        def load_xq_chunk(c):
            xt = xqp.tile([128, 8, 512], BF16D, name=f"xq{c}", tag="xqc")
            nc.sync.dma_start(
                out=xt,
                in_=xqT[:, c * 512:(c + 1) * 512].rearrange("(n p) m -> p n m", p=128))
            xq_t[c] = xt

        xk_t = []
        for half in range(2):
            xh = xkp.tile([128, 4, seq], BF16D, name=f"xk{half}", tag="xk")
            nc.sync.dma_start(
                out=xh, in_=xkT[half * 512:(half + 1) * 512].rearrange(
                    "(n p) m -> p n m", p=128))
            xk_t.append(xh)
            if half == 0:
                load_xq_chunk(0)

        # nm eighths: tile [128, 2(t), 2(h), 512]; SP queue, issued inside the
        # slot stream so the ring wait always lands on long-finished muls.
        nm_tiles = {}   # (u, i) -> tile

        def fetch_nm_eighth(u, i):
            upr, uc = u // 4, u % 4
            t0 = 2 * i
            nmq = nmp.tile([128, 2, 2, 512], BF16D, name="nmq", tag="nmq")
            nc.sync.dma_start(
                out=nmq, in_=nm[upr, uc, t0:t0 + 2].rearrange("t p h j -> p t h j"))
            nm_tiles[(u, i)] = nmq

        # ---------- filler chains as ~430ns PE quanta ----------
        # The tile framework encodes deps as per-engine counting semaphores,
        # so each exp effectively waits for ALL PE work emitted before its
        # score pair. Fillers are therefore emitted as small fixed-cost
        # quanta paced so every slot carries a uniform PE load.
        kt_t = {}
        qt_t = {}
        vpr_t = {}

        def KT(dt, q):
            # 4 quanta of 2 N=512 matmuls; Pool bias-add rides on the last
            st = {}
            cols = slice(q * 512, (q + 1) * 512)

            def part(i):
                def f():
                    if i == 0:
                        if dt not in kt_t:
                            kt_t[dt] = ktp.tile([128, seq], BF16D,
                                                name=f"kt{dt}", tag="kt")
                        st["ps"] = genps.tile([128, 512], F32, name="pps", tag="pps")
                    for db in (2 * i, 2 * i + 1):
                        nc.tensor.matmul(
                            st["ps"], wq_sb[:, db, dt * 128:(dt + 1) * 128],
                            xk_t[db // 4][:, db % 4, cols], start=(db == 0), stop=(db == 7))
                    if i == 3:
                        nc.vector.tensor_scalar_add(
                            kt_t[dt][:, cols], st["ps"], bq_sb[:, dt:dt + 1])
                return f
            return [(430, part(i)) for i in range(4)]

        def QT(dt, c):
            st = {}
            cols = slice(c * 512, (c + 1) * 512)

            def part(i):
                def f():
                    if i == 0:
                        if dt not in qt_t:
                            qt_t[dt] = qtp.tile([128, seq], BF16D,
                                                name=f"qt{dt}", tag="qt")
                        st["ps"] = genps.tile([128, 512], F32, name="pps", tag="pps")
                    for db in (2 * i, 2 * i + 1):
                        nc.tensor.matmul(
                            st["ps"], wq_sb[:, db, dt * 128:(dt + 1) * 128],
                            xq_t[c][:, db, :], start=(db == 0), stop=(db == 7))
                    if i == 3:
                        nc.vector.tensor_scalar_add(
                            qt_t[dt][:, cols], st["ps"], bq_sb[:, dt:dt + 1])
                return f
            return [(430, part(i)) for i in range(4)]

        def V(st_, pr):
            def f():
                if pr not in vpr_t:
                    vpr_t[pr] = vprp.tile([128, NT, 2, DH + 1], BF16D,
                                          name=f"vpr{pr}", tag="vpr")
                    nc.vector.memset(vpr_t[pr][:, :, :, DH:DH + 1], 1.0)
                ps = genps.tile([128, 512], F32, name="pps", tag="pps")
                for db in range(8):
                    nc.tensor.matmul(ps[:, 0:128],
                                     xv_t[db][:, st_ * 128:(st_ + 1) * 128],
                                     wq_sb[:, db, pr * 128:(pr + 1) * 128],
                                     start=(db == 0), stop=(db == 7))
                nc.vector.tensor_add(
                    vpr_t[pr][:, st_, :, 0:DH],
                    ps[:, 0:128].rearrange("p (h e) -> p h e", h=2),
                    bqrep[:, 2 * pr:2 * pr + 2, :])
            return [(430, f)]

        wo_holder = {}

        def OP(c, j):
            k, dch = j // 2, j % 2
            row = c * 512 + k * 128
            st = {}

            def part(i):
                def f():
                    if i == 0:
                        st["ps"] = genps.tile([128, 512], F32, name="pps", tag="pps")
                    for pr in (2 * i, 2 * i + 1):
                        nc.tensor.matmul(st["ps"], attnT[:, pr, row:row + 128],
                                         wo_holder["wo_t"][:, 2 * pr + dch, :],
                                         start=(pr == 0), stop=(pr == 3))
                    if i == 1:
                        osb = osbp.tile([128, 512], BF16D, name="osb", tag="osb")
                        nc.vector.tensor_copy(osb, st["ps"])
                        nc.sync.dma_start(
                            out=out[row:row + 128, dch * 512:(dch + 1) * 512],
                            in_=osb)
                return f
            return [(430, part(0)), (430, part(1))]

        # ---------- attention pipeline pieces ----------
        probs_store = {}  # u -> list of 16 probs tiles
        acc_store = {}    # u -> (acc0, acc1)

        def attn_chain(u, ci):
            def f():
                h, sub = ci // 4, ci % 4
                acch = acc_store[u][h]
                pr = u // 4
                for t in range(NT):
                    nc.tensor.matmul(
                        acch[:, sub, :],
                        probs_store[u][t][:, h, sub * 128:(sub + 1) * 128],
                        vpr_t[pr][:, t, h, :],
                        start=(t == 0), stop=(t == NT - 1))
            return [(440, f)]

        def finish_unit(u):
            pr, c = u // 4, u % 4
            cs = slice(c * 512, (c + 1) * 512)
            st = {}

            def recip():
                acc0, acc1 = acc_store[u]
                dens = densp.tile([128, 8], F32, name="dens", tag="dens")
                nc.vector.tensor_copy(
                    dens[:, 0:4], acc0[:, :, DH:DH + 1].rearrange("p a b -> p (a b)"))
                nc.vector.tensor_copy(
                    dens[:, 4:8], acc1[:, :, DH:DH + 1].rearrange("p a b -> p (a b)"))
                junk = densp.tile([128, 8], F32, name="junk", tag="junk")
                for j in range(8):
                    nc.gpsimd.normalize_recip(
                        junk[:, j:j + 1], dens[:, j:j + 1], dens[:, j:j + 1])
                st["dens"] = dens
                st["attn_n"] = attnnp.tile([128, 4, 2, DH], F32,
                                           name="attn_n", tag="attn_n")
                st["trans"] = genps.tile([128, 512], F32, name="pps", tag="pps")

            def norm(h):
                def f():
                    acch = acc_store[u][h]
                    for sub in range(4):
                        nc.vector.tensor_scalar_mul(
                            st["attn_n"][:, sub, h, :], acch[:, sub, 0:DH],
                            st["dens"][:, 4 * h + sub:4 * h + sub + 1])
                return f

            def trans4():
                # one transpose per sq-subtile covering both heads:
                # [128 sq, 128 d-pair] -> [128, 128] at PSUM partition 0
                # (HW requires transpose outputs to start at partition 0)
                tr4 = st["trans"][:, :].rearrange("p (a b) -> p a b", a=4)
                for sub in range(4):
                    nc.tensor.transpose(
                        tr4[:, sub, :],
                        st["attn_n"][:, sub, :, :].rearrange("p h e -> p (h e)"),
                        ident)

            def copy_out():
                nc.vector.tensor_copy(attnT[:, pr, cs], st["trans"])
                del probs_store[u]
            return [(60, recip), (40, norm(0)), (40, norm(1)),
                    (430, trans4), (60, copy_out)]

        # ---------- per-unit filler schedules (lists of quanta) ----------
        def qsum(*gens):
            out_ = []
            for g in gens:
                out_ += g
            return out_

        PRO_QT = QT(0, 0)
        PRO_KT = KT(0, 0)
        POST = {
            0: qsum(KT(0, 1), KT(0, 2), KT(0, 3))
               + [(cst, f, 9) for cst, f in QT(0, 1)]
               + [(cst, f, 10) for g in [V(s, 0) for s in range(16)] for cst, f in g],
            1: qsum(KT(1, 0)) + [(cst, f, 9) for cst, f in QT(0, 2)],
            2: qsum(KT(1, 1), KT(1, 2), KT(1, 3))
               + [(cst, f, 9) for cst, f in QT(0, 3)],
            3: qsum(QT(1, 0), *[V(s, 1) for s in range(8)]),
            4: qsum(KT(2, 0), KT(2, 1), QT(1, 1), *[V(s, 1) for s in range(8, 16)]),
            5: qsum(KT(2, 2), KT(2, 3), QT(1, 2)),
            6: qsum(QT(1, 3), *[V(s, 2) for s in range(8)]),
            7: qsum(QT(2, 0), *[V(s, 2) for s in range(8, 16)]),
            8: qsum(KT(3, 0), KT(3, 1), QT(2, 1)),
            9: qsum(KT(3, 2), KT(3, 3), QT(2, 2)),
            10: qsum(QT(2, 3), *[V(s, 3) for s in range(8)]),
            11: qsum(QT(3, 0), *[V(s, 3) for s in range(8, 16)]),
            12: qsum(QT(3, 1)),
            13: qsum(QT(3, 2), *[OP(0, j) for j in range(8)]),
            14: qsum(QT(3, 3), *[OP(1, j) for j in range(8)]),
            15: qsum(*[OP(2, j) for j in range(8)]),
        }
        TAIL = qsum(*[OP(3, j) for j in range(8)])

        # ---------- main loop ----------
        # xv immediately after xk/xq so u0's V fillers are never DMA-blocked
        xv_t = []
        for db in range(8):
            xt = xvp.tile([128, seq], BF16D, name=f"xv{db}", tag="xv")
            nc.sync.dma_start(out=xt, in_=xvT[db * 128:(db + 1) * 128, :])
            xv_t.append(xt)
        fetch_nm_eighth(0, 0)
        fetch_nm_eighth(0, 1)
        fetch_nm_eighth(0, 2)
        # Warm the PE p-state (full clock needs ~3us of continuous busy) and
        # keep it busy until each DMA lands; results are never read. Uses only
        # the dt0 columns of wq (the first wq DMA). The first K/Q chains are
        # interleaved with the warmup bursts to match xq-c0/xk-a/xk-b arrival.
        def warm(n):
            for w in range(n):
                wps = genps.tile([128, 512], F32, name="pps", tag="pps")
                nc.tensor.matmul(wps[:, 0:128], wq_sb[:, 0, 0:128],
                                 wq_sb[:, w % 8, 0:128], start=True, stop=True)
        warm(165)
        for _, f in PRO_QT:
            f()
        for _, f in PRO_KT[:2]:
            f()
        warm(85)
        for _, f in PRO_KT[2:]:
            f()

        for u in range(16):
            pr, c = u // 4, u % 4
            cs = slice(c * 512, (c + 1) * 512)
            acc0 = accp.tile([128, 4, DH + 1], F32, name="acc0", tag="acc0")
            acc1 = accp.tile([128, 4, DH + 1], F32, name="acc1", tag="acc1")
            acc_store[u] = (acc0, acc1)
            probs_store[u] = [None] * NT

            extras = []
            if u > 0:
                for ci in range(8):
                    extras += attn_chain(u - 1, ci)
                extras += finish_unit(u - 1)
            extras += POST[u]
            extras = [x if len(x) == 3 else (x[0], x[1], 0) for x in extras]
            total_cost = sum(x[0] for x in extras) or 1
            done_cost = 0
            n_x = 0
            for t in range(NT):
                spair = spairp.tile([128, 2, 512], F32, name="spair", tag="spair")
                tcol = slice(t * 128, (t + 1) * 128)
                nc.tensor.matmul(
                    spair[:, 0, :], kt_t[pr][0:64, tcol], qt_t[pr][0:64, cs],
                    start=True, stop=True, tile_position=(0, 0))
                nc.tensor.matmul(
                    spair[:, 1, :], kt_t[pr][64:128, tcol], qt_t[pr][64:128, cs],
                    start=True, stop=True, tile_position=(64, 0))
                probs = probsp.tile([128, 2, 512], BF16D, name="probs", tag="probs")
                nc.scalar.activation(probs, spair, EXP, scale=0.125)
                if t == 8 and u in (0, 1, 2):
                    load_xq_chunk(u + 1)
                if t == 14 and u == 0:
                    load_wq_cols(1)
                if t == 2 and u == 2:
                    load_wq_cols(2)
                if t == 2 and u == 5:
                    load_wq_cols(3)
                if t == 10 and u == 11:
                    # wo rides in the recycled xq-c0 buffer; the ring orders its
                    # DMA after the last QT(*, 0) reads.
                    wt = xqp.tile([128, 8, 512], BF16D, name="wo_t", tag="xqc")
                    _wap = wo[:, :]
                    nc.sync.dma_start(
                        out=wt[:, :, :].rearrange("p (n d) b -> p n d b", n=4),
                        in_=bass.AP(tensor=_wap.tensor, offset=_wap.offset,
                                    ap=[[1024, 128], [131072, 4], [512, 2],
                                        [1, 512]]))
                    wo_holder["wo_t"] = wt
                # prefetch nm eighth (prologue pre-loads e0-e2 of u0)
                if t % 2 == 1:
                    i_next = (t + 5) // 2
                    nu, ni = (u, i_next) if i_next < 8 else (u + 1, i_next - 8)
                    if nu < 16:
                        fetch_nm_eighth(nu, ni)
                nmq = nm_tiles[(u, t // 2)]
                if u == 0:
                    # u0's nm stream lands late (behind the x loads); Pool
                    # muls keep the DVE movers un-gated during warm-up
                    nc.gpsimd.tensor_mul(probs, probs, nmq[:, t % 2])
                else:
                    nc.vector.tensor_mul(probs, probs, nmq[:, t % 2])
                probs_store[u][t] = probs
                # pace extras by ns budget: counting-sem position semantics
                # make exp pace == PE pace through everything emitted before
                # each score pair, so keep per-slot PE load uniform.
                budget = total_cost * (t + 1) // NT
                while (n_x < len(extras) and done_cost < budget
                       and extras[n_x][2] <= t):
                    done_cost += extras[n_x][0]
                    extras[n_x][1]()
                    n_x += 1
            while n_x < len(extras):
                done_cost += extras[n_x][0]
                extras[n_x][1]()
                n_x += 1

        # tail: last unit's chains + finish + last outproj
        for ci in range(8):
            for _, f in attn_chain(15, ci):
                f()
        for _, f in finish_unit(15):
            f()
        for _, f in TAIL:
            f()

    nc.compile()
    return nc


_NC_CACHE = {}


def _get_nc(seq=SEQ):
    if seq not in _NC_CACHE:
        _NC_CACHE[seq] = build_nc(seq)
    return _NC_CACHE[seq]


def make_core_inputs(q, k, v, mask, W_q, b_q, W_o, seq=SEQ):
    """Build the 8 per-core input maps (host-side shard + layout)."""
    NT = seq // 128
    NC_ = seq // 512
    in_maps = []
    notm_all = (~np.asarray(mask)).astype(BF16)  # [B, 16, sq, sk]
    ident = np.eye(128, dtype=np.float32)
    for core in range(8):
        b, gi = divmod(core, 2)
        cols = slice(gi * DG, (gi + 1) * DG)
        xqT = np.ascontiguousarray(np.asarray(q[b], np.float32).T).astype(BF16)
        xkT = np.ascontiguousarray(np.asarray(k[b], np.float32).T).astype(BF16)
        xvT = np.ascontiguousarray(np.asarray(v[b], np.float32).T).astype(BF16)
        wqc = np.ascontiguousarray(np.asarray(W_q, np.float32)[:, cols]).astype(BF16)
        bqc = np.ascontiguousarray(np.asarray(b_q, np.float32)[cols])
        woc = np.ascontiguousarray(np.asarray(W_o, np.float32)[cols, :]).astype(BF16)
        nmc = notm_all[b, gi * HG:(gi + 1) * HG]  # [8, sq, sk] bf16
        # heads (pair, h) x [sq, sk] -> [pair, c, t, p(sk), h, j(sq)]
        nmc = np.ascontiguousarray(
            nmc.reshape(4, 2, NC_, 512, NT, 128).transpose(0, 2, 4, 5, 1, 3)
        )
        in_maps.append({
            "xqT": xqT, "xkT": xkT, "xvT": xvT,
            "wq": wqc, "bq": bqc, "wo": woc, "nm": nmc, "ident_in": ident,
        })
    return in_maps


def kernel(q, k, v, mask, W_q, b_q, W_o, b_o):
    nc = _get_nc(SEQ)
    in_maps = make_core_inputs(q, k, v, mask, W_q, b_q, W_o, SEQ)
    res = run_bass_kernel_spmd(nc, in_maps, core_ids=list(range(8)))
    out = np.empty((BATCH, SEQ, D_MODEL), np.float32)
    bo = np.asarray(b_o, np.float32)
    for b in range(BATCH):
        out[b] = (res.results[2 * b]["out"].astype(np.float32)
                  + res.results[2 * b + 1]["out"].astype(np.float32) + bo)
    return out
